# revision 8
# baseline (speedup 1.0000x reference)
"""Trainium2 Bass kernel for nn_ATKT (embed -> LSTM -> causal cumulative
attention -> FC+sigmoid).  Pure data parallel over batch: 64 sequences
sharded 8 per NeuronCore; parameters replicated.

Device layout notes (per core, B=8 sequences, S=512 steps):
  * tokens are t-major: token = t*8 + b  (4096 tokens)
  * "T-layout": feature dim on partitions, (…, b, t) on the free dim.
  * LSTM gate dim is host-permuted to [f, i, o, g] (256 each) so that
    sigmoid covers one contiguous slice and (f,i)x(c,g) products pair up.
  * hseqT  [128, 2, 8, S]   : h_t  (hidden half, b, t)
  * xqT    [128, 8, 64, 8]  : per-64-step chunk of Wih@x + b (m-tile, t, b)
  * gates PSUM [128, 64]    : (m-tile, b)
"""

import sys
import os
import numpy as np

if "/opt/trn_rl_repo" not in sys.path:
    sys.path.insert(0, "/opt/trn_rl_repo")

P = 128
B_TOT, S, NCORES = 64, 512, 8
BC = B_TOT // NCORES            # 8 sequences per core
H, G, EIN = 256, 1024, 512      # hidden, 4*hidden, lstm input
NCLS, AD = 1000, 80             # fc classes, attention dim
CHUNK_T = 64                    # LSTM-input chunk, in time steps
NCHUNK = S // CHUNK_T
TOK = S * BC                    # tokens per core (4096)
TT = TOK // P                   # token tiles (32)
CHUNK_TOK = CHUNK_T * BC        # tokens per chunk (512)

# dtype knobs
WHH_BF16 = True      # Whh stationary tiles bf16 (FWL weight loads)
HSEQ_BF16 = True     # h stored bf16 (must match WHH dtype for the matmul)
XQ_BF16 = False      # xproj chunk buffers

_CACHE = {}


def _gate_perm():
    # torch order i,f,g,o -> device order f,i,o,g
    return np.r_[256:512, 0:256, 768:1024, 512:768]


def _build_program(s_steps=S):
    import concourse.bass as bass
    import concourse.mybir as mybir
    import concourse.tile as tile
    from concourse import bacc

    f32 = mybir.dt.float32
    f32r = mybir.dt.float32r
    bf16 = mybir.dt.bfloat16
    i32 = mybir.dt.int32
    FT = mybir.ActivationFunctionType
    OP = mybir.AluOpType

    hseq_dt = bf16 if HSEQ_BF16 else f32
    whh_dt = bf16 if WHH_BF16 else f32
    xq_dt = bf16 if XQ_BF16 else f32
    nchunk = s_steps // CHUNK_T
    tok = s_steps * BC
    tt = tok // P

    def mmcast(ap):
        # fp32 matmuls run at 1/4 rate; fp32r is full rate for N>=256.
        return ap.bitcast(f32r) if ap.dtype == f32 else ap

    nc = bacc.Bacc(None, target_bir_lowering=False)

    # ---- DRAM I/O ----------------------------------------------------
    aug_emb = nc.dram_tensor("aug_emb", [1003, 256], f32, kind="ExternalInput")
    idx1 = nc.dram_tensor("idx1", [P, tt], i32, kind="ExternalInput")
    idx2 = nc.dram_tensor("idx2", [P, tt], i32, kind="ExternalInput")
    wih_d = nc.dram_tensor("wihT", [P, 4 * G], f32r, kind="ExternalInput")
    whh_d = nc.dram_tensor("whhT", [P, 2 * G], whh_dt, kind="ExternalInput")
    bias_d = nc.dram_tensor("biasT", [P, 8], f32, kind="ExternalInput")
    mlp_d = nc.dram_tensor("mlpT", [P, 2 * AD], hseq_dt if HSEQ_BF16 else f32r, kind="ExternalInput")
    mlpb_d = nc.dram_tensor("mlpb", [AD, 1], f32, kind="ExternalInput")
    simr_d = nc.dram_tensor("simrep", [AD, P], f32r, kind="ExternalInput")
    negm_d = nc.dram_tensor("negm", [P, 1], f32, kind="ExternalInput")
    fcw_d = nc.dram_tensor("fcwT", [P, 4 * NCLS], hseq_dt, kind="ExternalInput")
    fcb_d = nc.dram_tensor("fcb", [1, NCLS], f32r, kind="ExternalInput")
    ident_d = nc.dram_tensor("ident", [P, P], f32, kind="ExternalInput")
    ones_d = nc.dram_tensor("ones1", [1, P], f32r, kind="ExternalInput")

    res_d = nc.dram_tensor("res_out", [BC, s_steps, NCLS], f32, kind="ExternalOutput")
    sa_d = nc.dram_tensor("sa_out", [BC, s_steps, EIN], f32, kind="ExternalOutput")
    DBG = bool(os.environ.get("BASSDBG"))
    if DBG:
        dbg_h = nc.dram_tensor("dbg_h", [P, 2, BC, s_steps], hseq_dt, kind="ExternalOutput")
        dbg_w = nc.dram_tensor("dbg_w", [P, BC, s_steps], f32, kind="ExternalOutput")
        dbg_x = nc.dram_tensor("dbg_x", [P, 8, CHUNK_T, BC], xq_dt, kind="ExternalOutput")
        dbg_c = nc.dram_tensor("dbg_c", [P, 2, s_steps], f32, kind="ExternalOutput")

    def sa_rows(j):
        # token tile j rows (t-major): partitions p = (t_local*8 + b)
        return sa_d[:, j * 16:(j + 1) * 16, :].rearrange("b s e -> s b e")

    with tile.TileContext(nc) as tc:
        cpool = tc.tile_pool(name="consts", bufs=1)
        with cpool as cp:
            wih_sb = cp.tile([P, 4, G], f32r)
            whh_sb = cp.tile([P, 2, G], whh_dt)
            bias_sb = cp.tile([P, 8], f32)
            mlp_sb = cp.tile([P, 2, AD], hseq_dt if HSEQ_BF16 else f32r)
            mlpb_sb = cp.tile([AD, 1], f32)
            simr_sb = cp.tile([AD, P], f32r)
            negm_sb = cp.tile([P, 1], f32)
            fcw_sb = cp.tile([P, 4, NCLS], hseq_dt)
            fcb_sb = cp.tile([1, NCLS], f32r)
            iden_sb = cp.tile([P, P], f32)
            ones_sb = cp.tile([1, P], f32r)
            idx1_sb = cp.tile([P, tt], i32)
            idx2_sb = cp.tile([P, tt], i32)

            nc.sync.dma_start(out=wih_sb[:], in_=wih_d.rearrange("p (k g) -> p k g", k=4))
            nc.sync.dma_start(out=whh_sb[:], in_=whh_d.rearrange("p (k g) -> p k g", k=2))
            nc.sync.dma_start(out=bias_sb[:], in_=bias_d[:])
            nc.sync.dma_start(out=mlp_sb[:], in_=mlp_d.rearrange("p (k a) -> p k a", k=2))
            nc.sync.dma_start(out=mlpb_sb[:], in_=mlpb_d[:])
            nc.sync.dma_start(out=simr_sb[:], in_=simr_d[:])
            nc.sync.dma_start(out=negm_sb[:], in_=negm_d[:])
            nc.sync.dma_start(out=fcw_sb[:], in_=fcw_d.rearrange("p (k n) -> p k n", k=4))
            nc.sync.dma_start(out=fcb_sb[:], in_=fcb_d[:])
            nc.sync.dma_start(out=iden_sb[:], in_=ident_d[:])
            nc.sync.dma_start(out=ones_sb[:], in_=ones_d[:])
            nc.sync.dma_start(out=idx1_sb[:], in_=idx1[:])
            nc.sync.dma_start(out=idx2_sb[:], in_=idx2[:])

            # persistent state
            spool = tc.tile_pool(name="state", bufs=1)
            with spool as sp:
                hseq = sp.tile([P, 2, BC, s_steps], hseq_dt)
                pre_sb = sp.tile([P, 64], f32)
                s_sb = sp.tile([P, 48], f32)
                cg_sb = sp.tile([P, 32], f32)   # [c(16) | g(16)]
                prod_sb = sp.tile([P, 32], f32)
                tc_sb = sp.tile([P, 16], f32)
                nc.vector.memzero(cg_sb[:, 0:16])

                def lstm_chunk(ci, xq):
                    t0 = ci * CHUNK_T
                    for dt_ in range(CHUNK_T):
                        t = t0 + dt_
                        xsl = xq[:, :, dt_, :]  # [P, 8, 8]
                        if t == 0:
                            nc.vector.tensor_copy(
                                pre_sb[:].rearrange("p (m b) -> p m b", m=8), xsl)
                        else:
                            gps = g_psum.tile([P, 64], f32)
                            for m in range(8):
                                for k in range(2):
                                    nc.tensor.matmul(
                                        gps[:, m * 8:(m + 1) * 8],
                                        whh_sb[:, k, m * P:(m + 1) * P],
                                        hseq[:, k, :, t - 1],
                                        start=(k == 0), stop=(k == 1))
                            nc.vector.tensor_add(
                                pre_sb[:].rearrange("p (m b) -> p m b", m=8),
                                gps[:].rearrange("p (m b) -> p m b", m=8), xsl)
                        nc.scalar.activation(s_sb[:, 0:48], pre_sb[:, 0:48], FT.Sigmoid)
                        nc.scalar.activation(cg_sb[:, 16:32], pre_sb[:, 48:64], FT.Tanh)
                        nc.vector.tensor_mul(prod_sb[:], s_sb[:, 0:32], cg_sb[:])
                        nc.vector.tensor_add(cg_sb[:, 0:16], prod_sb[:, 0:16], prod_sb[:, 16:32])
                        nc.scalar.activation(tc_sb[:], cg_sb[:, 0:16], FT.Tanh)
                        nc.vector.tensor_mul(
                            hseq[:, :, :, t],
                            s_sb[:, 32:48].rearrange("p (k b) -> p k b", k=2),
                            tc_sb[:].rearrange("p (k b) -> p k b", k=2))

                # ---- phase 1 (per chunk): gather -> sa_out -> transpose
                # -> xproj matmul; interleaved with LSTM of previous chunk
                with tc.tile_pool(name="gat", bufs=6) as gat_pool, \
                     tc.tile_pool(name="saT", bufs=2) as saT_pool, \
                     tc.tile_pool(name="xq", bufs=2) as xq_pool, \
                     tc.tile_pool(name="trps", bufs=2, space="PSUM") as tr_psum, \
                     tc.tile_pool(name="xps", bufs=2, space="PSUM") as x_psum, \
                     tc.tile_pool(name="gps", bufs=1, space="PSUM") as g_psum:

                    xq_tiles = []
                    for ci in range(nchunk):
                        saT = saT_pool.tile([P, 4, CHUNK_TOK], f32r)
                        for jj in range(4):  # token tiles within chunk
                            j = ci * 4 + jj
                            sa_t = gat_pool.tile([P, EIN], f32)
                            nc.gpsimd.indirect_dma_start(
                                out=sa_t[:, 0:256], out_offset=None,
                                in_=aug_emb[:],
                                in_offset=bass.IndirectOffsetOnAxis(
                                    ap=idx1_sb[:, j:j + 1], axis=0))
                            nc.gpsimd.indirect_dma_start(
                                out=sa_t[:, 256:512], out_offset=None,
                                in_=aug_emb[:],
                                in_offset=bass.IndirectOffsetOnAxis(
                                    ap=idx2_sb[:, j:j + 1], axis=0))
                            nc.sync.dma_start(out=sa_rows(j), in_=sa_t[:])
                            for et in range(4):
                                trp = tr_psum.tile([P, P], f32)
                                nc.tensor.transpose(
                                    trp[:], sa_t[:, et * P:(et + 1) * P], iden_sb[:])
                                nc.scalar.copy(
                                    saT[:, et, jj * P:(jj + 1) * P], trp[:])
                        xq = xq_pool.tile([P, 8, CHUNK_T, BC], xq_dt)
                        xq_tiles.append(xq)
                        if DBG and ci == 0:
                            pass
                        for m in range(8):
                            xps = x_psum.tile([P, CHUNK_TOK], f32)
                            for k in range(4):
                                nc.tensor.matmul(
                                    xps[:], wih_sb[:, k, m * P:(m + 1) * P],
                                    saT[:, k, :],
                                    start=(k == 0), stop=(k == 3))
                            nc.scalar.activation(
                                xq[:, m, :, :].rearrange("p t b -> p (t b)"),
                                xps[:], FT.Identity, bias=bias_sb[:, m:m + 1])
                        if DBG and ci == 0:
                            nc.sync.dma_start(out=dbg_x[:], in_=xq[:])
                        if ci >= 1:
                            lstm_chunk(ci - 1, xq_tiles[ci - 1])
                    lstm_chunk(nchunk - 1, xq_tiles[nchunk - 1])
                    if DBG:
                        nc.sync.dma_start(out=dbg_h[:], in_=hseq[:])

                # ---- phase 2: attention + FC, streamed per sequence b --
                with tc.tile_pool(name="att", bufs=2) as att_pool, \
                     tc.tile_pool(name="attb", bufs=2) as attb_pool, \
                     tc.tile_pool(name="res", bufs=3) as res_pool, \
                     tc.tile_pool(name="mps", bufs=2, space="PSUM") as m_psum, \
                     tc.tile_pool(name="aps", bufs=2, space="PSUM") as a_psum, \
                     tc.tile_pool(name="fps", bufs=2, space="PSUM") as f_psum:

                    mT = att_pool.tile([AD, BC, s_steps], f32r, tag="mT", bufs=1)
                    wrep = att_pool.tile([P, BC, s_steps], f32, tag="wrep", bufs=1)
                    for b in range(BC):
                        mp = m_psum.tile([AD, s_steps], f32)
                        for k in range(2):
                            nc.tensor.matmul(
                                mp[:], mlp_sb[:, k, :],
                                hseq[:, k, b, :],
                                start=(k == 0), stop=(k == 1))
                        nc.scalar.activation(mT[:, b, :], mp[:], FT.Tanh,
                                             bias=mlpb_sb[:, 0:1])
                        ap_ = a_psum.tile([P, s_steps], f32)
                        nc.tensor.matmul(ap_[:], simr_sb[:],
                                         mT[:, b, :], start=True, stop=True)
                        nc.scalar.activation(wrep[:, b, :], ap_[:], FT.Exp,
                                             bias=negm_sb[:, 0:1])
                    if DBG:
                        nc.sync.dma_start(out=dbg_w[:], in_=wrep[:])

                    for b in range(BC):
                        den = attb_pool.tile([P, s_steps], f32, tag="den")
                        nc.vector.tensor_tensor_scan(
                            den[:], wrep[:, b, :], wrep[:, b, :], 0.0,
                            op0=OP.add, op1=OP.bypass)
                        rden = attb_pool.tile([P, s_steps], f32, tag="rden")
                        nc.vector.reciprocal(rden[:], den[:])
                        wh = attb_pool.tile([P, 2, s_steps], f32, tag="wh")
                        num = attb_pool.tile([P, 2, s_steps], f32, tag="num")
                        ao = attb_pool.tile([P, 2, s_steps], f32, tag="ao")
                        cum1 = attb_pool.tile([P, 2, s_steps], hseq_dt, tag="cum1")
                        for hf in range(2):
                            nc.vector.tensor_mul(wh[:, hf, :], hseq[:, hf, b, :],
                                                 wrep[:, b, :])
                            nc.vector.tensor_tensor_scan(
                                num[:, hf, :], wh[:, hf, :], wh[:, hf, :], 0.0,
                                op0=OP.add, op1=OP.bypass)
                            nc.vector.tensor_mul(ao[:, hf, :], num[:, hf, :], rden[:])
                            # inclusive cumsum of ao -> wh (reuse), then
                            # exclusive = inclusive - ao
                            nc.vector.tensor_tensor_scan(
                                wh[:, hf, :], ao[:, hf, :], ao[:, hf, :], 0.0,
                                op0=OP.add, op1=OP.bypass)
                            nc.vector.tensor_sub(cum1[:, hf, :], wh[:, hf, :],
                                                 ao[:, hf, :])
                            if DBG and b == 0:
                                nc.sync.dma_start(out=dbg_c[:, hf, :], in_=ao[:, hf, :])
                        MT = min(P, s_steps)
                        for mt in range(s_steps // MT):
                            res_t = res_pool.tile([MT, NCLS], f32)
                            for nb in range(2):
                                nsl = slice(nb * 500, (nb + 1) * 500)
                                fp = f_psum.tile([MT, 500], f32)
                                nc.tensor.matmul(
                                    fp[:], ones_sb[:, 0:MT], fcb_sb[:, nsl],
                                    start=True, stop=False)
                                for k in range(4):
                                    lhsT = (cum1[:, k, mt * MT:(mt + 1) * MT] if k < 2
                                            else hseq[:, k - 2, b, mt * MT:(mt + 1) * MT])
                                    nc.tensor.matmul(
                                        fp[:], lhsT, fcw_sb[:, k, nsl],
                                        start=False, stop=(k == 3))
                                nc.scalar.activation(res_t[:, nsl], fp[:], FT.Sigmoid)
                            nc.sync.dma_start(
                                out=res_d[b, mt * MT:(mt + 1) * MT, :], in_=res_t[:])

    nc.compile()
    return nc


def _host_prep(inputs):
    import ml_dtypes
    perm = _gate_perm()
    skill = np.asarray(inputs["skill"]).astype(np.int32)
    answer = np.asarray(inputs["answer"]).astype(np.int32)
    skill_emb = np.asarray(inputs["skill_emb"], dtype=np.float32)
    answer_emb = np.asarray(inputs["answer_emb"], dtype=np.float32)
    Wih = np.asarray(inputs["Wih"], dtype=np.float32)
    Whh = np.asarray(inputs["Whh"], dtype=np.float32)
    bias = (np.asarray(inputs["bih"], dtype=np.float32)
            + np.asarray(inputs["bhh"], dtype=np.float32))
    mlp_W = np.asarray(inputs["mlp_W"], dtype=np.float32)
    mlp_b = np.asarray(inputs["mlp_b"], dtype=np.float32)
    sim_W = np.asarray(inputs["sim_W"], dtype=np.float32)
    fc_W = np.asarray(inputs["fc_W"], dtype=np.float32)
    fc_b = np.asarray(inputs["fc_b"], dtype=np.float32)

    aug = np.concatenate([skill_emb, answer_emb[0:1], answer_emb[1:2]], 0)
    aug = np.ascontiguousarray(aug, dtype=np.float32)

    wihT = Wih.T[:, perm]                       # [512, 1024]
    wihT_r = np.ascontiguousarray(
        wihT.reshape(4, P, G).transpose(1, 0, 2).reshape(P, 4 * G))
    whhT = Whh.T[:, perm]                       # [256, 1024]
    whh_r = np.ascontiguousarray(
        whhT.reshape(2, P, G).transpose(1, 0, 2).reshape(P, 2 * G))
    whh_r = whh_r.astype(ml_dtypes.bfloat16) if WHH_BF16 else whh_r
    biasT = np.ascontiguousarray(bias[perm].reshape(8, P).T)  # [128, 8]
    mlpT = mlp_W.T                              # [256, 80]
    mlpT_r = np.ascontiguousarray(
        mlpT.reshape(2, P, AD).transpose(1, 0, 2).reshape(P, 2 * AD))
    if HSEQ_BF16:
        mlpT_r = mlpT_r.astype(ml_dtypes.bfloat16)
    mlpb_r = np.ascontiguousarray(mlp_b.reshape(AD, 1))
    simrep = np.ascontiguousarray(np.tile(sim_W[0][:, None], (1, P)))
    negm = np.full((P, 1), -np.abs(sim_W).sum(), np.float32)
    fcwT = fc_W.T                               # [512, 1000]
    fcw_r = np.ascontiguousarray(
        fcwT.reshape(4, P, NCLS).transpose(1, 0, 2).reshape(P, 4 * NCLS))
    fcw_r = fcw_r.astype(ml_dtypes.bfloat16) if HSEQ_BF16 else fcw_r
    fcb_r = np.ascontiguousarray(fc_b.reshape(1, NCLS))
    ident = np.eye(P, dtype=np.float32)
    ones1 = np.ones((1, P), np.float32)

    in_maps = []
    for core in range(NCORES):
        sl = slice(core * BC, (core + 1) * BC)
        sk, an = skill[sl], answer[sl]          # [8, S]
        i1 = np.where(an == 1, sk, 1001).T.reshape(TOK)   # t-major
        i2 = np.where(an == 1, 1002, sk).T.reshape(TOK)
        in_maps.append({
            "aug_emb": aug,
            "idx1": np.ascontiguousarray(i1.reshape(TT, P).T.astype(np.int32)),
            "idx2": np.ascontiguousarray(i2.reshape(TT, P).T.astype(np.int32)),
            "wihT": wihT_r, "whhT": whh_r, "biasT": biasT,
            "mlpT": mlpT_r, "mlpb": mlpb_r, "simrep": simrep, "negm": negm,
            "fcwT": fcw_r, "fcb": fcb_r, "ident": ident, "ones1": ones1,
        })
    return in_maps


def kernel(**inputs):
    from concourse.bass_utils import run_bass_kernel_spmd
    if "prog" not in _CACHE:
        _CACHE["prog"] = _build_program()
    nc = _CACHE["prog"]
    in_maps = _host_prep(inputs)
    trace = bool(os.environ.get("BASS_TRACE_RUN"))
    out = run_bass_kernel_spmd(nc, in_maps, list(range(NCORES)), trace=trace,
                               tmpdir=os.environ.get("BASS_TRACE_DIR"))
    _CACHE["last"] = out
    res = np.concatenate([r["res_out"] for r in out.results], 0)
    sa = np.concatenate([r["sa_out"] for r in out.results], 0)
    return res.astype(np.float32), sa.astype(np.float32)
